# revision 28
# baseline (speedup 1.0000x reference)
"""Trainium2 Bass kernel: pre-LN multi-head attention block (B=8, L=1024,
D=1024, H=16, dk=dv=64), data-parallel over batch across 8 NeuronCores.

v2 design (all-bf16 PE path):
  - k, v pre-cast to bf16 on host; q stays fp32 (LN + residual precision).
  - pre-LN gamma/beta folded into Wq on host: Q = xhat @ wq' + bq, with the
    bias added during the PSUM->SBUF evacuation (per-partition AP scalar).
  - x^T built by PE transposes in bf16 (1 cyc/row), 8 blocks per psum tile,
    single strided DVE evac per token tile.
  - S^T per head via 64-partition-offset matmul operands (no zero-padding).
  - exp on ACT reads a [128,1024] psum tile (2 banks / 2 matmul groups).
  - PV with ones-augmented V (sumexp rides along as psum row 64); lane-64
    reciprocal + 1-partition matmul broadcast; no sumexp DMAs.
  - output projection flipped token-major (stationary O^T chunks, moving Wo
    rows): no output transposes; residual add + LN + store pipelined per
    token tile.
"""

import numpy as np
import ml_dtypes

import concourse.bass as bass
import concourse.mybir as mybir
import concourse.tile as tile
from concourse import bacc
from concourse.dve_ops import RECIP_APPROX_FAST_CONSTS, RECIPROCAL_APPROX_FAST

P = 128
L = 1024          # tokens per batch element
D = 1024          # model dim
H = 16            # heads
HD = 64           # head dim
E = HD + 1        # head dim + sumexp column
NC = D // P       # 8 feature chunks
NT = L // P       # 8 token chunks
NQ = 2            # 512-wide halves of the moving/free dimension
QH = 512
EPS = 1e-6

FP32 = mybir.dt.float32
BF16 = mybir.dt.bfloat16
FP32R = mybir.dt.float32r
OP = mybir.AluOpType
AF = mybir.ActivationFunctionType


def _emit_ln_stats(nc, pool, x, scratch, eps_t):
    """Return (rstd, neg_mu_rstd) per-partition [P,1] APs for LN of x."""
    st = pool.tile([P, 8], FP32, tag="lnst", bufs=4, name="lnst")
    nc.scalar.activation(scratch, x, AF.Copy, accum_out=st[:, 0:1])
    nc.scalar.activation(scratch, x, AF.Square, accum_out=st[:, 1:2])
    nc.vector.tensor_scalar_mul(st[:, 2:3], st[:, 0:1], 1.0 / D)     # mu
    nc.vector.tensor_tensor(st[:, 3:4], st[:, 2:3], st[:, 2:3], OP.mult)
    nc.vector.tensor_scalar_mul(st[:, 4:5], st[:, 1:2], 1.0 / D)     # E[x^2]
    nc.vector.tensor_tensor(st[:, 4:5], st[:, 4:5], st[:, 3:4], OP.subtract)
    nc.scalar.activation(st[:, 5:6], st[:, 4:5], AF.Sqrt, bias=eps_t)
    nc.vector.reciprocal(st[:, 6:7], st[:, 5:6])                     # rstd
    nc.vector.tensor_tensor(st[:, 7:8], st[:, 2:3], st[:, 6:7], OP.mult)
    nc.vector.tensor_scalar_mul(st[:, 7:8], st[:, 7:8], -1.0)        # -mu*rstd
    return st[:, 6:7], st[:, 7:8]


def build_bass():
    nc = bacc.Bacc("TRN2", target_bir_lowering=False, debug=False)

    qb_d = nc.dram_tensor("qb", [L, D], BF16, kind="ExternalInput")
    kb_d = nc.dram_tensor("kb", [L, D], BF16, kind="ExternalInput")
    vb_d = nc.dram_tensor("vb", [L, D], BF16, kind="ExternalInput")
    wq_d = nc.dram_tensor("wq", [D, D], BF16, kind="ExternalInput")
    wk_d = nc.dram_tensor("wk", [D, D], BF16, kind="ExternalInput")
    wv_d = nc.dram_tensor("wv", [D, D], BF16, kind="ExternalInput")
    wo_d = nc.dram_tensor("wo", [D, D], BF16, kind="ExternalInput")
    bq_d = nc.dram_tensor("bq", [P, NC], FP32, kind="ExternalInput")
    gb_d = nc.dram_tensor("gb", [P, D], FP32, kind="ExternalInput")
    bb_d = nc.dram_tensor("bb", [P, D], FP32, kind="ExternalInput")
    id_d = nc.dram_tensor("ident", [P, P], BF16, kind="ExternalInput")
    on_d = nc.dram_tensor("ones64", [P, HD], FP32R, kind="ExternalInput")
    ep_d = nc.dram_tensor("epsc", [P, 1], FP32, kind="ExternalInput")
    vo_d = nc.dram_tensor("vone", [P, H * E], BF16, kind="ExternalInput")
    out_d = nc.dram_tensor("out", [L, D], FP32, kind="ExternalOutput")

    with tile.TileContext(nc) as tc:
        with tc.tile_pool(name="persist", bufs=1) as pp:
            ident = pp.tile([P, P], BF16, name="ident")
            eps_t = pp.tile([P, 1], FP32, name="eps_t")
            ones64 = pp.tile([P, HD], FP32R, name="ones64")
            bq_t = pp.tile([P, NC], FP32, name="bq_t")
            KT = pp.tile([P, NC, L], BF16, name="KT")
            QT = pp.tile([P, NC, L], BF16, name="QT")
            Vaug = pp.tile([P, NT, H * E], BF16, name="Vaug")
            OT = [pp.tile([P, L], BF16, name=f"ot{j}") for j in range(H // 2)]
            qb = [pp.tile([P, D], BF16, name=f"qb{t}") for t in range(NT)]

            nc.sync.dma_start(ident, id_d[:])
            nc.sync.dma_start(eps_t, ep_d[:])
            nc.sync.dma_start(ones64, on_d[:])
            nc.sync.dma_start(bq_t, bq_d[:])

            # ---------------- QKV phase ----------------
            with (
                tc.tile_pool(name="qkv", bufs=1) as qp,
                tc.tile_pool(name="psA", bufs=1, space="PSUM") as psA,
            ):
                def load_w(dram, nm):
                    tiles = []
                    for i in range(NC):
                        wt = qp.tile([P, D], BF16, tag="w", bufs=2 * NC,
                                     name=f"w{nm}{i}")
                        nc.sync.dma_start(wt, dram[i * P:(i + 1) * P, :])
                        tiles.append(wt)
                    return tiles

                def transpose_tile(dst, x, t):
                    """dst[:, c, t*128:+128] = x[:, c*128:+128]^T for all c."""
                    pt = psA.tile([P, D], BF16, tag="tr", bufs=2, name="ps_tr")
                    for c in range(NC):
                        nc.tensor.transpose(
                            pt[:, c * P:(c + 1) * P],
                            x[:, c * P:(c + 1) * P], ident)
                    nc.vector.tensor_copy(
                        dst[:, :, t * P:(t + 1) * P],
                        pt.rearrange("p (c x) -> p c x", x=P))

                def xT_tile():
                    return qp.tile([P, NC, L], BF16, tag="xT", bufs=2,
                                   name="xT")

                # ---- k -> kT -> K-proj ----
                # kb DMAs go first on the sync queue; tile 0 is split into
                # column chunks across DMA queues so the first transpose can
                # start within ~2us.
                kT = xT_tile()
                k0c = []
                for c in range(NC):
                    xc = qp.tile([P, P], BF16, tag="k0c", bufs=NC,
                                 name=f"k0c{c}")
                    nc.sync.dma_start(xc, kb_d[0:P, c * P:(c + 1) * P])
                    k0c.append(xc)
                kin = [None]
                for t in range(1, NT):
                    x = qp.tile([P, D], BF16, tag="kin", bufs=4,
                                name="k_in")
                    nc.sync.dma_start(x, kb_d[t * P:(t + 1) * P, :])
                    kin.append(x)
                wk_t = load_w(wk_d, "k")

                # prefetch v tiles + ones columns + Wv while K-proj runs
                vin = []
                for t in range(NT):
                    x = qp.tile([P, D], BF16, tag="vin", bufs=NT, name="v_in")
                    nc.sync.dma_start(x, vb_d[t * P:(t + 1) * P, :])
                    vin.append(x)
                vo1 = qp.tile([P, H * E], BF16, name="vo1")
                nc.sync.dma_start(vo1, vo_d[:])
                for t in range(NT):
                    nc.gpsimd.dma_start(Vaug[:, t, :], vo1)

                # q DMA + LN emitted early: the ACT-side LN overlaps the
                # k transposes / K-proj so the qn transposes are not gated.
                # qb stays resident in bf16 and doubles as the residual.
                qn = []
                for t in range(NT):
                    nc.sync.dma_start(qb[t], qb_d[t * P:(t + 1) * P, :])
                    y = qp.tile([P, D], BF16, tag="qn", bufs=NT, name="qn")
                    rstd, nmr = _emit_ln_stats(nc, qp, qb[t], y, eps_t)
                    nc.scalar.activation(y, qb[t], AF.Identity, bias=nmr,
                                         scale=rstd)
                    qn.append(y)
                wv_t = load_w(wv_d, "v")

                pt0 = psA.tile([P, D], BF16, tag="tr", bufs=2,
                               name="ps_tr")
                for c in range(NC):
                    nc.tensor.transpose(pt0[:, c * P:(c + 1) * P],
                                        k0c[c], ident)
                nc.vector.tensor_copy(
                    kT[:, :, 0:P], pt0.rearrange("p (c x) -> p c x", x=P))
                for t in range(1, NT):
                    transpose_tile(kT, kin[t], t)

                def proj_feat(w_tiles, src, dst, bias_col=None):
                    for m in range(NC):
                        ps = psA.tile([P, L], FP32, tag="pj", bufs=2,
                                      name="ps_pj")
                        for n in range(NQ):
                            for i in range(NC):
                                nc.tensor.matmul(
                                    ps[:, n * QH:(n + 1) * QH],
                                    w_tiles[i][:, m * P:(m + 1) * P],
                                    src[:, i, n * QH:(n + 1) * QH],
                                    start=(i == 0), stop=(i == NC - 1))
                        if bias_col is None:
                            nc.vector.tensor_copy(dst[:, m, :], ps)
                        else:
                            nc.vector.tensor_scalar_add(
                                dst[:, m, :], ps, bias_col[:, m:m + 1])

                proj_feat(wk_t, kT, KT)

                # ---- v -> vT -> V-proj (token-major, into Vaug) ----
                vT = xT_tile()
                for t in range(NT):
                    transpose_tile(vT, vin[t], t)
                wq_t = load_w(wq_d, "q")
                for t in range(NT):
                    ps = psA.tile([P, L], FP32, tag="pj", bufs=2, name="ps_v")
                    for n in range(NQ):
                        for i in range(NC):
                            nc.tensor.matmul(
                                ps[:, n * QH:(n + 1) * QH],
                                vT[:, i, t * P:(t + 1) * P],
                                wv_t[i][:, n * QH:(n + 1) * QH],
                                start=(i == 0), stop=(i == NC - 1))
                    dst = Vaug[:, t, :].rearrange("p (h e) -> p h e", e=E)
                    nc.vector.tensor_copy(
                        dst[:, :, 0:HD],
                        ps.rearrange("p (h x) -> p h x", x=HD))

                # ---- qn -> qnT -> Q-proj (bias folded) ----
                qnT = xT_tile()
                for t in range(NT):
                    transpose_tile(qnT, qn[t], t)
                proj_feat(wq_t, qnT, QT, bias_col=bq_t)

            # ---------------- out-phase inputs (emit DMAs early) ----------
            with tc.tile_pool(name="fin", bufs=1) as fp:
                gamma_bc = fp.tile([P, D], FP32, name="gamma_bc")
                beta_bc = fp.tile([P, D], FP32, name="beta_bc")
                nc.sync.dma_start(gamma_bc, gb_d[:])
                nc.sync.dma_start(beta_bc, bb_d[:])
                wo_t = []
                for j in range(NC):
                    wt = fp.tile([P, D], BF16, tag="wo", bufs=NC,
                                 name=f"wo{j}")
                    nc.sync.dma_start(wt, wo_d[j * P:(j + 1) * P, :])
                    wo_t.append(wt)


                # ---------------- attention ----------------
                with (
                    tc.tile_pool(name="att", bufs=1) as ap,
                    tc.tile_pool(name="psS", bufs=2, space="PSUM") as psS,
                    tc.tile_pool(name="psO", bufs=3, space="PSUM") as psO,
                    tc.tile_pool(name="psB", bufs=1, space="PSUM") as psB,
                ):
                    rc = RECIP_APPROX_FAST_CONSTS

                    def emit_pv(h, PT):
                        """PV matmuls for head h (consumes PT)."""
                        pos = []
                        for n in range(NQ):
                            po = psO.tile([E, QH], FP32, tag="o",
                                          name="ps_o")
                            pos.append(po)
                        for i in range(NT):
                            for n in range(NQ):
                                nc.tensor.matmul(
                                    pos[n],
                                    Vaug[:, i, h * E:(h + 1) * E],
                                    PT[:, i, n * QH:(n + 1) * QH],
                                    start=(i == 0), stop=(i == NT - 1))
                        return pos

                    def epilogue_a(h, pos):
                        """Drain PV psums to SBUF + kick off the reciprocal
                        chain (no PE work; frees the psO slots fast).

                        The approx-fast DVE reciprocal only works at
                        partition 0, so the sumexp row goes psum(row 64)
                        -> sbuf(lane 64) -> DMA -> sbuf(lane 0).
                        """
                        rin = ap.tile([E, L], FP32, tag="rin", bufs=2,
                                      name="rin")
                        rec = ap.tile([1, L], FP32R, tag="rec", bufs=2,
                                      name="rec")
                        ou = ap.tile([HD, L], FP32, tag="ou", bufs=2,
                                     name="ou")
                        for n in range(NQ):
                            ns = slice(n * QH, (n + 1) * QH)
                            nc.vector.tensor_copy(rin[HD:E, ns],
                                                  pos[n][HD:E, :])
                            nc.vector.tensor_copy(ou[:, ns], pos[n][0:HD, :])
                        nc.gpsimd.dma_start(rin[0:1, :], rin[HD:E, :])
                        nc.vector._custom_dve(
                            RECIPROCAL_APPROX_FAST, out=rec, in0=rin[0:1, :],
                            s0=rc["s0"], s1=rc["s1"], imm2=rc["imm2"])
                        return (h, ou, rec)

                    def epilogue_b(h, ou, rec):
                        """Broadcast 1/sumexp via the PE and scale into OT.
                        Emitted a block after epilogue_a so the PE never
                        waits on the reciprocal chain."""
                        c, half = h // 2, h % 2
                        otmp = None
                        if half == 1:
                            otmp = ap.tile([HD, L], BF16, tag="otmp",
                                           bufs=2, name="otmp")
                        for n in range(NQ):
                            ns = slice(n * QH, (n + 1) * QH)
                            pb = psB.tile([HD, QH], FP32, tag="b",
                                          name="ps_b")
                            nc.tensor.matmul(pb, ones64[0:1, :],
                                             rec[0:1, ns],
                                             start=True, stop=True)
                            if half == 0:
                                nc.vector.tensor_tensor(
                                    OT[c][0:HD, ns], ou[:, ns], pb, OP.mult)
                            else:
                                nc.vector.tensor_tensor(
                                    otmp[:, ns], ou[:, ns], pb, OP.mult)
                        if half == 1:
                            nc.gpsimd.dma_start(OT[c][HD:P, :], otmp)

                    prev = None  # (h, PT) pending PV
                    epi = None   # (h, ou, rec) pending epilogue_b
                    for h in range(H):
                        c, half = h // 2, h % 2
                        hs = slice(half * HD, half * HD + HD)
                        PT = ap.tile([P, NC, L], BF16, tag="pt", bufs=2,
                                     name="pt")
                        # S matmuls + exp for head h, with PV of head h-1
                        # interleaved chunk-by-chunk to keep the PE dense
                        pv_pos = None
                        if prev is not None:
                            ph, pPT = prev
                            pv_pos = [psO.tile([E, QH], FP32, tag="o",
                                               name="ps_o")
                                      for _ in range(NQ)]
                        for i in range(NT):
                            ks = slice(i * P, (i + 1) * P)
                            ss = psS.tile([P, L], FP32, tag="s", name="ps_s")
                            for n in range(NQ):
                                nc.tensor.matmul(
                                    ss[:, n * QH:(n + 1) * QH],
                                    KT[hs, c, ks],
                                    QT[hs, c, n * QH:(n + 1) * QH],
                                    start=True, stop=True)
                            nc.scalar.activation(PT[:, i, :], ss, AF.Exp)
                            if prev is not None:
                                for n in range(NQ):
                                    nc.tensor.matmul(
                                        pv_pos[n],
                                        Vaug[:, i, ph * E:(ph + 1) * E],
                                        pPT[:, i, n * QH:(n + 1) * QH],
                                        start=(i == 0), stop=(i == NT - 1))
                            if i == 3 and epi is not None:
                                epilogue_b(*epi)
                                epi = None
                        if prev is not None:
                            epi = epilogue_a(prev[0], pv_pos)
                        prev = (h, PT)
                    # drain: PV + epilogues of the last heads
                    if epi is not None:
                        epilogue_b(*epi)
                    pos = emit_pv(prev[0], prev[1])
                    epi = epilogue_a(prev[0], pos)
                    epilogue_b(*epi)

                # ------------- output projection + residual + LN ---------
                # Software-pipelined two deep: tile t's j=0..6 matmuls run
                # before tile t-1's j=7 (so the last head pair's OT has
                # time to land), and the LN chain trails by another tile.
                # Residual-add + row-sum fused in one DVE STT; beta-add on
                # the otherwise idle GpSimd.
                with tc.tile_pool(name="psW", bufs=3, space="PSUM") as psW:
                    def emit_j7_u(t, ps):
                        for n in range(NQ):
                            nc.tensor.matmul(
                                ps[:, n * QH:(n + 1) * QH],
                                OT[NC - 1][:, t * P:(t + 1) * P],
                                wo_t[NC - 1][:, n * QH:(n + 1) * QH],
                                start=False, stop=True)
                        u = fp.tile([P, D], FP32, tag="u", bufs=3, name="u")
                        st = fp.tile([P, 8], FP32, tag="lnst", bufs=3,
                                     name="lnst")
                        nc.vector.scalar_tensor_tensor(
                            u, ps, 0.0, qb[t], OP.add, OP.add,
                            accum_out=st[:, 0:1])
                        return (t, u, st)

                    def emit_ln_out(t, u, st):
                        """Mostly-DVE LN: one cross-engine hop (the tiny
                        sqrt), apply via two STT ops:
                        z = ((u - mu) * gamma) * rstd + beta."""
                        y = fp.tile([P, D], FP32, tag="y", bufs=3, name="y")
                        nc.scalar.activation(y, u, AF.Square,
                                             accum_out=st[:, 1:2])
                        nc.vector.tensor_scalar_mul(st[:, 2:3], st[:, 0:1],
                                                    1.0 / D)
                        nc.vector.tensor_tensor(st[:, 3:4], st[:, 2:3],
                                                st[:, 2:3], OP.mult)
                        nc.vector.tensor_scalar_mul(st[:, 4:5], st[:, 1:2],
                                                    1.0 / D)
                        nc.vector.tensor_tensor(st[:, 4:5], st[:, 4:5],
                                                st[:, 3:4], OP.subtract)
                        nc.scalar.activation(st[:, 5:6], st[:, 4:5],
                                             AF.Sqrt, bias=eps_t)
                        nc.vector.reciprocal(st[:, 6:7], st[:, 5:6])
                        nc.vector.scalar_tensor_tensor(
                            y, u, st[:, 2:3], gamma_bc,
                            OP.subtract, OP.mult)
                        z = fp.tile([P, D], FP32, tag="z", bufs=3, name="z")
                        nc.vector.scalar_tensor_tensor(
                            z, y, st[:, 6:7], beta_bc, OP.mult, OP.add)
                        nc.sync.dma_start(out_d[t * P:(t + 1) * P, :], z)

                    open_ps = {}
                    lnq = []
                    for t in range(NT):
                        ps = psW.tile([P, D], FP32, tag="w", name="ps_w")
                        for n in range(NQ):
                            for j in range(NC - 1):
                                nc.tensor.matmul(
                                    ps[:, n * QH:(n + 1) * QH],
                                    OT[j][:, t * P:(t + 1) * P],
                                    wo_t[j][:, n * QH:(n + 1) * QH],
                                    start=(j == 0), stop=False)
                        open_ps[t] = ps
                        if t - 1 in open_ps:
                            lnq.append(emit_j7_u(t - 1, open_ps.pop(t - 1)))
                        if len(lnq) > 1:
                            emit_ln_out(*lnq.pop(0))
                    lnq.append(emit_j7_u(NT - 1, open_ps.pop(NT - 1)))
                    for args in lnq:
                        emit_ln_out(*args)

    nc.compile()
    return nc


_CACHE = {}


def _get_nc():
    if "nc" not in _CACHE:
        _CACHE["nc"] = build_bass()
    return _CACHE["nc"]


def make_in_maps(q, k, v, Wq, Wk, Wv, Wo, gamma, beta):
    qb = np.asarray(q, np.float32).astype(ml_dtypes.bfloat16)
    kb = np.asarray(k, np.float32).astype(ml_dtypes.bfloat16)
    vb = np.asarray(v, np.float32).astype(ml_dtypes.bfloat16)
    gamma = np.asarray(gamma, np.float32)
    beta = np.asarray(beta, np.float32)
    Wq = np.asarray(Wq, np.float32)
    # fold pre-LN gamma/beta and the 1/sqrt(dk)=0.125 scale into Wq
    wq = (0.125 * gamma[:, None] * Wq).astype(ml_dtypes.bfloat16)
    bq = (0.125 * (beta @ Wq)).astype(np.float32)           # [D]
    bq_t = np.ascontiguousarray(bq.reshape(NC, P).T)        # [P, NC]
    wk = np.asarray(Wk, np.float32).astype(ml_dtypes.bfloat16)
    wv = np.asarray(Wv, np.float32).astype(ml_dtypes.bfloat16)
    wo = np.asarray(Wo, np.float32).astype(ml_dtypes.bfloat16)
    gb = np.ascontiguousarray(np.tile(gamma[None, :], (P, 1)))
    bb = np.ascontiguousarray(np.tile(beta[None, :], (P, 1)))
    ident = np.eye(P, dtype=np.float32).astype(ml_dtypes.bfloat16)
    ones64 = np.ones((P, HD), np.float32)
    epsc = np.full((P, 1), EPS, np.float32)
    vone = np.ones((P, H * E), ml_dtypes.bfloat16)
    B = q.shape[0]
    return [
        {
            "qb": np.ascontiguousarray(qb[b]),
            "kb": np.ascontiguousarray(kb[b]),
            "vb": np.ascontiguousarray(vb[b]),
            "wq": wq, "wk": wk, "wv": wv, "wo": wo, "bq": bq_t,
            "gb": gb, "bb": bb, "ident": ident, "ones64": ones64,
            "epsc": epsc, "vone": vone,
        }
        for b in range(B)
    ]


def kernel(q, k, v, Wq, Wk, Wv, Wo, gamma, beta, trace=False):
    from concourse.bass_utils import run_bass_kernel_spmd

    nc = _get_nc()
    in_maps = make_in_maps(q, k, v, Wq, Wk, Wv, Wo, gamma, beta)
    res = run_bass_kernel_spmd(nc, in_maps, core_ids=list(range(len(in_maps))),
                               trace=trace)
    out = np.stack([r["out"] for r in res.results], axis=0)
    if trace:
        return out, res
    return out
